# revision 1
# baseline (speedup 1.0000x reference)
"""CrossATT kernel for Trainium2 (Bass/Tile), data-parallel over batch on 8 cores.

Math (per batch b):
    S = x_cont @ x_ques^T            # [C, Q]
    A = softmax(S, axis=-1)          # over q
    c2q = A @ x_ques                 # [C, D]
    out = c2q @ W1 + x_cont @ W0     # [C, D]

Device-side formulation works fully transposed so the TensorE contraction
axis is always on partitions and softmax needs no on-chip transposes.
W1 is folded into x_ques on the host (QW = x_ques @ W1) and the W0 term
(x_cont @ W0, no attention dependence) is added on the host, so the device
computes only:
    ST[q, c]  = sum_d QT[d, q] * XT[d, c]         (MM1, per 128-q chunk)
    E         = exp(ST)                           (no max subtraction: |S| < ~70)
    s[c]      = sum_q E[q, c]                     (DVE/GPSIMD partial adds + ones-matmul)
    OT[e, c]  = (sum_q QW[q, e] * E[q, c]) / s[c] (MM2 + reciprocal broadcast mul)
host:
    out = OT^T + x_cont @ W0

All device matmuls run in float32r (TF32-class: ~1.5e-4 rel err, 1 cycle/row
at moving width >= 256 vs fp32's 4). The reciprocal row broadcast runs on the
otherwise-idle GPSIMD engine (partition_broadcast), which also takes one of
the three partial-sum adds to offload DVE.
"""

import os

import numpy as np

import concourse.bass as bass
import concourse.mybir as mybir
import concourse.tile as tile
from concourse import bacc, library_config
from concourse.bass_utils import run_bass_kernel_spmd

B, C_LEN, Q_LEN, D = 16, 4096, 512, 128
NCORES = 8
BPC = B // NCORES          # batches per core
CB = 512                   # c-block width (PSUM bank / max f32 moving width)
NBLK = C_LEN // CB         # 8 blocks per batch
NQ = Q_LEN // 128          # 4 q-chunks

F32R = mybir.dt.float32r
F32 = mybir.dt.float32

_CACHE = {}


def _build():
    nc = bacc.Bacc("TRN2", target_bir_lowering=False, debug=False, num_devices=NCORES)

    XT = nc.declare_dram_parameter("XT", [BPC, D, C_LEN], F32R, isOutput=False)
    QT = nc.declare_dram_parameter("QT", [BPC, D, Q_LEN], F32R, isOutput=False)
    QW = nc.declare_dram_parameter("QW", [BPC, Q_LEN, D], F32R, isOutput=False)
    OT = nc.declare_dram_parameter("OT", [BPC, D, C_LEN], F32, isOutput=True)

    # Timing-only knobs. KREPEAT>1 repeats the computation inside one NEFF;
    # per-block 4-byte checksum outputs keep every repeat's compute alive
    # (dead-store elim would otherwise drop repeats that rewrite OT).
    # KCHAIN=1 makes each repeat data-depend on the previous one so the
    # slope measures serial per-kernel latency instead of pipelined
    # throughput.
    repeat = int(os.environ.get("KREPEAT", "1"))
    chain = int(os.environ.get("KCHAIN", "0"))
    CS = None
    if repeat > 1:
        CS = nc.declare_dram_parameter(
            "CS", [repeat * BPC, NBLK], F32, isOutput=True
        )

    with tile.TileContext(nc) as tc:
        with (
            tc.tile_pool(name="const", bufs=1) as const,
            tc.tile_pool(name="xt", bufs=4) as xtp,
            tc.tile_pool(name="e", bufs=4) as ep,
            tc.tile_pool(name="padd", bufs=2) as paddp,
            tc.tile_pool(name="r", bufs=2) as rp,
            tc.tile_pool(name="rbc", bufs=2) as rbcp,
            tc.tile_pool(name="osb", bufs=3) as osbp,
            tc.tile_pool(name="ps_st", bufs=2, space="PSUM") as ps_st,
            tc.tile_pool(name="ps_s", bufs=2, space="PSUM") as ps_s,
            tc.tile_pool(name="ps_o", bufs=2, space="PSUM") as ps_o,
        ):
            nc.gpsimd.load_library(library_config.proxy)

            # column of 128 ones: stationary for the sums matmul
            ones_f = const.tile([128, 1], F32)
            nc.vector.memset(ones_f, 1.0)
            ones_r = const.tile([128, 1], F32R)
            nc.vector.tensor_copy(out=ones_r, in_=ones_f)

            qt_sb = []
            qw_sb = []
            for b in range(BPC):
                qt = const.tile([D, Q_LEN], F32R, name=f"qt{b}")
                nc.sync.dma_start(out=qt, in_=QT[b])
                qt_sb.append(qt)
                qw = const.tile([128, NQ, D], F32R, name=f"qw{b}")
                nc.sync.dma_start(
                    out=qw, in_=QW[b].rearrange("(k p) d -> p k d", p=128)
                )
                qw_sb.append(qw)

            chain_src = None
            for b_rep in range(repeat * BPC):
                b = b_rep % BPC
                qt_cur = qt_sb[b]
                if chain and chain_src is not None:
                    # serialize this repeat behind the previous one
                    qt_cur = paddp.tile([D, Q_LEN], F32R, tag="qtmod")
                    with nc.allow_low_precision(reason="timing-only chain"):
                        nc.vector.tensor_scalar_add(
                            out=qt_cur, in0=qt_sb[b], scalar1=chain_src[:, 0:1]
                        )
                for j in range(NBLK):
                    cs = bass.ts(j, CB)
                    xt_blk = xtp.tile([D, CB], F32R, tag="xt")
                    nc.sync.dma_start(out=xt_blk, in_=XT[b][:, cs])

                    # MM1 into paired PSUM tiles; one exp per pair (halves
                    # the 352-cycle ACTIVATE fixed overhead)
                    e_pairs = []
                    for h in range(NQ // 2):
                        st = ps_st.tile([128, 2, CB], F32, tag="st")
                        for i in range(2):
                            k = 2 * h + i
                            nc.tensor.matmul(
                                out=st[:, i, :],
                                lhsT=qt_cur[:, bass.ts(k, 128)],
                                rhs=xt_blk,
                                start=True,
                                stop=True,
                            )
                        e = ep.tile([128, 2, CB], F32R, tag="e")
                        nc.scalar.activation(
                            out=e, in_=st, func=mybir.ActivationFunctionType.Exp
                        )
                        e_pairs.append(e)

                    # partial sums over q chunks (DVE + one on GPSIMD),
                    # then ones-matmul -> s[1, CB]
                    p01 = paddp.tile([128, CB], F32R, tag="p01")
                    nc.vector.tensor_add(
                        out=p01, in0=e_pairs[0][:, 0, :], in1=e_pairs[0][:, 1, :]
                    )
                    p23 = paddp.tile([128, CB], F32R, tag="p23")
                    nc.gpsimd.tensor_add(
                        out=p23, in0=e_pairs[1][:, 0, :], in1=e_pairs[1][:, 1, :]
                    )
                    part = paddp.tile([128, CB], F32R, tag="part")
                    nc.vector.tensor_add(out=part, in0=p01, in1=p23)

                    s_ps = ps_s.tile([1, CB], F32)
                    nc.tensor.matmul(
                        out=s_ps, lhsT=ones_r, rhs=part, start=True, stop=True
                    )
                    r_sb = rp.tile([1, CB], F32)
                    nc.vector.reciprocal(out=r_sb, in_=s_ps)

                    # broadcast recip row across partitions on GPSIMD
                    r_bc = rbcp.tile([128, CB], F32)
                    nc.gpsimd.partition_broadcast(r_bc, r_sb)

                    # MM2: OT1 = QW^T E (unnormalized c2q@W1, transposed)
                    o_ps = ps_o.tile([D, CB], F32)
                    for h in range(NQ // 2):
                        for i in range(2):
                            k = 2 * h + i
                            nc.tensor.matmul(
                                out=o_ps,
                                lhsT=qw_sb[b][:, k, :],
                                rhs=e_pairs[h][:, i, :],
                                start=(k == 0),
                                stop=(k == NQ - 1),
                            )
                    # normalize while moving PSUM->SBUF, then store
                    o_sb = osbp.tile([D, CB], F32)
                    nc.vector.tensor_mul(out=o_sb, in0=o_ps, in1=r_bc)
                    if repeat == 1 or b_rep >= (repeat - 1) * BPC:
                        nc.sync.dma_start(out=OT[b][:, cs], in_=o_sb)
                    if CS is not None:
                        nc.sync.dma_start(
                            out=CS[b_rep : b_rep + 1, j : j + 1], in_=o_sb[0:1, 0:1]
                        )
                    if chain and j == NBLK - 1:
                        chain_src = o_sb

    nc.compile()
    return nc


def kernel(x_cont, x_ques, c_mask, q_mask, W0, W1):
    x_cont = np.ascontiguousarray(x_cont, dtype=np.float32)
    x_ques = np.ascontiguousarray(x_ques, dtype=np.float32)
    W0 = np.ascontiguousarray(W0, dtype=np.float32)
    W1 = np.ascontiguousarray(W1, dtype=np.float32)

    if "nc" not in _CACHE:
        _CACHE["nc"] = _build()
    nc = _CACHE["nc"]

    xt = np.ascontiguousarray(x_cont.transpose(0, 2, 1))  # [B, D, C]
    qt = np.ascontiguousarray(x_ques.transpose(0, 2, 1))  # [B, D, Q]
    qw = np.matmul(x_ques, W1)                            # [B, Q, D]

    in_maps = []
    for i in range(NCORES):
        sl = slice(i * BPC, (i + 1) * BPC)
        in_maps.append({"XT": xt[sl], "QT": qt[sl], "QW": qw[sl]})

    res = run_bass_kernel_spmd(nc, in_maps, core_ids=list(range(NCORES)))

    out = np.matmul(x_cont, W0)  # [B, C, D] — attention-free term, on host
    for i in range(NCORES):
        ot = res.results[i]["OT"]  # [BPC, D, C]
        out[i * BPC : (i + 1) * BPC] += ot.transpose(0, 2, 1)
    return out


# --- timing helper for test.py (not used by the graded kernel() path) ---
def timed_run(x_cont, x_ques, W0, W1, iters=10):
    """Persistent-jit execution; returns (list of wall times per exec, out).

    Replicates bass2jax.run_bass_via_pjrt but keeps the jitted callable and
    device-resident inputs across iterations so the measured time is
    dispatch + NEFF execution, not retracing/host transfers.
    """
    import time

    import jax
    from jax.sharding import Mesh, PartitionSpec
    from jax.experimental.shard_map import shard_map

    import concourse.mybir as _mybir
    from concourse import bass2jax

    if "nc" not in _CACHE:
        _CACHE["nc"] = _build()
    nc = _CACHE["nc"]
    bass2jax.install_neuronx_cc_hook()

    xt = np.ascontiguousarray(x_cont.transpose(0, 2, 1))
    qt = np.ascontiguousarray(x_ques.transpose(0, 2, 1))
    qw = np.matmul(x_ques, W1)
    full = {"XT": xt, "QT": qt, "QW": qw}

    partition_name = nc.partition_id_tensor.name if nc.partition_id_tensor else None
    in_names, out_names, out_avals, zero_outs = [], [], [], []
    for alloc in nc.m.functions[0].allocations:
        if not isinstance(alloc, _mybir.MemoryLocationSet):
            continue
        name = alloc.memorylocations[0].name
        if alloc.kind == "ExternalInput":
            if name != partition_name:
                in_names.append(name)
        elif alloc.kind == "ExternalOutput":
            shape = tuple(alloc.tensor_shape)
            dtype = _mybir.dt.np(alloc.dtype)
            out_names.append(name)
            out_avals.append(jax.core.ShapedArray(shape, dtype))
            zero_outs.append(np.zeros(shape, dtype))
    n_params = len(in_names)
    n_outs = len(out_avals)
    all_names = in_names + out_names
    if partition_name is not None:
        all_names = all_names + [partition_name]

    def _body(*args):
        operands = list(args)
        if partition_name is not None:
            operands.append(bass2jax.partition_id_tensor())
        outs = bass2jax._bass_exec_p.bind(
            *operands,
            out_avals=tuple(out_avals),
            in_names=tuple(all_names),
            out_names=tuple(out_names),
            lowering_input_output_aliases=(),
            sim_require_finite=True,
            sim_require_nnan=True,
            nc=nc,
        )
        return tuple(outs)

    devices = jax.devices()[:NCORES]
    mesh = Mesh(np.asarray(devices), ("core",))
    spec = PartitionSpec("core")
    donate = tuple(range(n_params, n_params + n_outs))
    sharded = jax.jit(
        shard_map(
            _body,
            mesh=mesh,
            in_specs=(spec,) * (n_params + n_outs),
            out_specs=(spec,) * n_outs,
            check_rep=False,
        ),
        donate_argnums=donate,
        keep_unused=True,
    )

    sharding = jax.sharding.NamedSharding(mesh, spec)
    concat_in = [
        jax.device_put(np.ascontiguousarray(full[name]), sharding)
        for name in in_names
    ]

    def fresh_zeros():
        return [
            jax.device_put(
                np.zeros((NCORES * z.shape[0], *z.shape[1:]), z.dtype), sharding
            )
            for z in zero_outs
        ]

    out_arrs = sharded(*concat_in, *fresh_zeros())
    jax.block_until_ready(out_arrs)

    zsets = [fresh_zeros() for _ in range(iters)]
    times = []
    for zs in zsets:
        t0 = time.perf_counter()
        out_arrs = sharded(*concat_in, *zs)
        jax.block_until_ready(out_arrs)
        times.append(time.perf_counter() - t0)
    return times, out_arrs



# revision 2
# speedup vs baseline: 1133.5458x; 1133.5458x over previous
"""CrossATT kernel for Trainium2 (Bass/Tile), data-parallel over batch on 8 cores.

Math (per batch b):
    S = x_cont @ x_ques^T            # [C, Q]
    A = softmax(S, axis=-1)          # over q
    c2q = A @ x_ques                 # [C, D]
    out = c2q @ W1 + x_cont @ W0     # [C, D]

Device-side formulation works fully transposed so the TensorE contraction
axis is always on partitions and softmax needs no on-chip transposes.
W1 is folded into x_ques on the host (QW = x_ques @ W1) and the W0 term
(x_cont @ W0, no attention dependence) is added on the host. The softmax
normalization (divide by the per-column sum s) also happens on the host:
shipping the unnormalized OT plus the [1, C] sums removes the serial
reciprocal -> partition_broadcast -> multiply drain chain that kept PSUM
banks alive and starved the PE (the [1,512] single-partition RECIPROCAL
alone was 3.3us per block on DVE).

Device computes, per 512-column c-block:
    ST[q, c]  = sum_d QT[d, q] * XT[d, c]      (MM1, fp16 in, f32 psum)
    E         = exp(ST) -> bf16                (no max subtraction: |S| < ~70,
                                                exp(S) < 3e29 fits f32/bf16)
    part      = pairwise-tree sum of the 4 q-chunks of E (2 DVE + 1 GPSIMD)
    s[1, c]   = ones^T @ part                  (ones-matmul, f32 psum)
    OT[e, c]  = sum_q QW[q, e] * E[q, c]       (MM2, bf16, f32 psum)
host:
    out = (OT / s)^T + x_cont @ W0

dtypes: MM1 runs on fp16 inputs — same 1 cycle/row PE speed as fp32r at
moving width 512, but half the HBM traffic, and fp16's 10-bit mantissa
matches fp32r precision (bf16 inputs to MM1 push the final rel err to
2.1e-2, over the 2e-2 gate; fp16 gives 3.5e-3). E/MM2/OT are bf16 (E
overflows fp16's 6.5e4 max; bf16 on MM2 costs ~2e-3). All matmuls are
1 cycle/row, so PE time is the 55us floor fixed by the math
(2*C*Q*D MACs/batch) plus ~7us for the s-matmuls.
"""

import numpy as np
import ml_dtypes

import concourse.bass as bass
import concourse.mybir as mybir
import concourse.tile as tile
from concourse import bacc, library_config
from concourse.bass_utils import run_bass_kernel_spmd

B, C_LEN, Q_LEN, D = 16, 4096, 512, 128
NCORES = 8
BPC = B // NCORES          # batches per core
CB = 512                   # c-block width (PSUM bank / max f32 moving width)
NBLK = C_LEN // CB         # 8 blocks per batch
NQ = Q_LEN // 128          # 4 q-chunks

F32 = mybir.dt.float32
F16 = mybir.dt.float16
BF16 = mybir.dt.bfloat16

_CACHE = {}


def _build():
    nc = bacc.Bacc("TRN2", target_bir_lowering=False, debug=False, num_devices=NCORES)

    XT = nc.declare_dram_parameter("XT", [BPC, D, C_LEN], F16, isOutput=False)
    QT = nc.declare_dram_parameter("QT", [BPC, D, Q_LEN], F16, isOutput=False)
    QW = nc.declare_dram_parameter("QW", [BPC, 128, NQ, D], BF16, isOutput=False)
    OT = nc.declare_dram_parameter("OT", [BPC, D, C_LEN], BF16, isOutput=True)
    SS = nc.declare_dram_parameter("SS", [BPC, NBLK, CB], F32, isOutput=True)

    with tile.TileContext(nc) as tc:
        with (
            tc.tile_pool(name="const", bufs=1) as const,
            tc.tile_pool(name="xt", bufs=2) as xtp,
            tc.tile_pool(name="e", bufs=4) as ep,
            tc.tile_pool(name="padd", bufs=6) as paddp,
            tc.tile_pool(name="ssb", bufs=2) as ssbp,
            tc.tile_pool(name="osb", bufs=3) as osbp,
            tc.tile_pool(name="ps_st", bufs=2, space="PSUM") as ps_st,
            tc.tile_pool(name="ps_s", bufs=2, space="PSUM") as ps_s,
            tc.tile_pool(name="ps_o", bufs=2, space="PSUM") as ps_o,
        ):
            nc.gpsimd.load_library(library_config.proxy)

            # column of 128 ones: stationary for the sums matmul
            ones_f = const.tile([128, 1], F32)
            nc.vector.memset(ones_f, 1.0)
            ones_b = const.tile([128, 1], BF16)
            nc.vector.tensor_copy(out=ones_b, in_=ones_f)

            qt_sb = []
            qw_sb = []
            for b in range(BPC):
                qt = const.tile([D, Q_LEN], F16, name=f"qt{b}")
                nc.sync.dma_start(out=qt, in_=QT[b])
                qt_sb.append(qt)
                qw = const.tile([128, NQ, D], BF16, name=f"qw{b}")
                nc.sync.dma_start(out=qw, in_=QW[b])
                qw_sb.append(qw)

            for b in range(BPC):
                # whole batch of XT in one DMA (1 MB, contiguous)
                xt_b = xtp.tile([D, C_LEN], F16, tag="xt")
                nc.sync.dma_start(out=xt_b, in_=XT[b])
                for j in range(NBLK):
                    cs = bass.ts(j, CB)

                    # MM1 into paired PSUM tiles; one exp per pair (halves
                    # the 352-cycle ACTIVATE fixed overhead)
                    e_pairs = []
                    for h in range(NQ // 2):
                        st = ps_st.tile([128, 2, CB], F32, tag="st")
                        for i in range(2):
                            k = 2 * h + i
                            nc.tensor.matmul(
                                out=st[:, i, :],
                                lhsT=qt_sb[b][:, bass.ts(k, 128)],
                                rhs=xt_b[:, cs],
                                start=True,
                                stop=True,
                            )
                        e = ep.tile([128, 2, CB], BF16, tag="e")
                        nc.scalar.activation(
                            out=e, in_=st, func=mybir.ActivationFunctionType.Exp
                        )
                        e_pairs.append(e)

                    # pairwise tree sum over the 4 q-chunks (bf16): two adds
                    # on DVE, one on the otherwise-idle GPSIMD
                    p01 = paddp.tile([128, CB], BF16, tag="p01")
                    nc.vector.tensor_add(
                        out=p01, in0=e_pairs[0][:, 0, :], in1=e_pairs[0][:, 1, :]
                    )
                    p23 = paddp.tile([128, CB], BF16, tag="p23")
                    nc.gpsimd.tensor_add(
                        out=p23, in0=e_pairs[1][:, 0, :], in1=e_pairs[1][:, 1, :]
                    )
                    part = paddp.tile([128, CB], BF16, tag="part")
                    nc.vector.tensor_add(out=part, in0=p01, in1=p23)

                    # s[1, c] = colsum(E) via ones-matmul; out to host for
                    # the normalization divide
                    s_ps = ps_s.tile([1, CB], F32)
                    nc.tensor.matmul(
                        out=s_ps, lhsT=ones_b, rhs=part, start=True, stop=True
                    )
                    s_sb = ssbp.tile([1, CB], F32, tag="s")
                    nc.vector.tensor_copy(out=s_sb, in_=s_ps)
                    nc.sync.dma_start(out=SS[b][j : j + 1, :], in_=s_sb)

                    # MM2: OT = QW^T E (unnormalized c2q@W1, transposed)
                    o_ps = ps_o.tile([D, CB], F32)
                    for k in range(NQ):
                        nc.tensor.matmul(
                            out=o_ps,
                            lhsT=qw_sb[b][:, k, :],
                            rhs=e_pairs[k // 2][:, k % 2, :],
                            start=(k == 0),
                            stop=(k == NQ - 1),
                        )
                    o_sb = osbp.tile([D, CB], BF16, tag="o")
                    nc.vector.tensor_copy(out=o_sb, in_=o_ps)
                    nc.sync.dma_start(out=OT[b][:, cs], in_=o_sb)

    nc.compile()
    return nc


def _prep_inmaps(x_cont, x_ques, W1):
    """Host-side shard + layout prep: returns per-core input maps."""
    xt = np.ascontiguousarray(
        x_cont.transpose(0, 2, 1), dtype=np.float16
    )  # [B, D, C] fp16
    qt = np.ascontiguousarray(
        x_ques.transpose(0, 2, 1), dtype=np.float16
    )  # [B, D, Q] fp16
    qw = np.matmul(x_ques, W1)  # [B, Q, D] f32
    # [B, Q, D] -> [B, 128, NQ, D] so the DMA is a straight copy
    qw = np.ascontiguousarray(
        qw.reshape(B, NQ, 128, D).transpose(0, 2, 1, 3)
    ).astype(ml_dtypes.bfloat16)

    in_maps = []
    for i in range(NCORES):
        sl = slice(i * BPC, (i + 1) * BPC)
        in_maps.append({"XT": xt[sl], "QT": qt[sl], "QW": qw[sl]})
    return in_maps


def _postprocess(x_cont, W0, results):
    """Gather per-core outputs, normalize, add the host-side W0 term."""
    out = np.matmul(x_cont, W0)  # [B, C, D] — attention-free term, on host
    for i in range(NCORES):
        ot = results[i]["OT"].astype(np.float32)  # [BPC, D, C]
        ss = results[i]["SS"].reshape(BPC, C_LEN)  # [BPC, C]
        out[i * BPC : (i + 1) * BPC] += (ot / ss[:, None, :]).transpose(0, 2, 1)
    return out


def kernel(x_cont, x_ques, c_mask, q_mask, W0, W1):
    x_cont = np.ascontiguousarray(x_cont, dtype=np.float32)
    x_ques = np.ascontiguousarray(x_ques, dtype=np.float32)
    W0 = np.ascontiguousarray(W0, dtype=np.float32)
    W1 = np.ascontiguousarray(W1, dtype=np.float32)

    if "nc" not in _CACHE:
        _CACHE["nc"] = _build()
    nc = _CACHE["nc"]

    in_maps = _prep_inmaps(x_cont, x_ques, W1)
    res = run_bass_kernel_spmd(nc, in_maps, core_ids=list(range(NCORES)))
    return _postprocess(x_cont, W0, res.results)
